# revision 17
# baseline (speedup 1.0000x reference)
"""Bidirectional LSTM + vocab projection kernel for 8 Trainium2 NeuronCores.

Per-core program (identical on all cores; only the fc_w shard input differs):
  - Embedding gather (indirect DMA) + PE transpose to x^T (E on partitions).
  - Both LSTM directions interleaved per step, with 4x column-tiled bf16
    matmuls for the gate GEMM; x@Wx is folded into the same PSUM
    accumulation (no xg precompute). Gates/cell state in fp32.
  - h^T (bf16) stays fully resident in SBUF and feeds the FC phase.
  - FC phase: out^T[vocab, token] = fc_w_shard^T @ h_cat, bf16 operands,
    fp32 accumulate; vocab sharded 8 ways (4096 padded columns per core).
  - Host assembles/transposes the final [B, T, V] fp32 output.

Token order on-device: column j = t*B + b (t-major, b-fast).
Recurrent weight column order: col = q*512 + g*128 + u (q = unit-group,
g = gate i/f/o/chat, u = unit-in-group) so each column-tile quarter holds
all four gates of one unit group. K-chunk order: r=0,1 -> x (E halves),
r=2..5 -> h unit-groups 0..3 (x first so next step's x matmuls can be
hoisted by the scheduler while the previous step's tail still runs).
"""

import numpy as np
from contextlib import ExitStack

import ml_dtypes
import concourse.bass as bass
import concourse.tile as tile
from concourse import bacc, mybir
from concourse.bass_utils import run_bass_kernel_spmd

N_CORES = 8
B, T, V, E, H = 16, 256, 32000, 256, 512
NTOK = B * T                      # 4096 tokens
VPAD = 4096                       # per-core padded vocab shard
VSH = V // N_CORES                # 4000 real vocab columns per core
G4 = 4 * H                        # 2048 gate columns
KR = 6                            # K chunks: 2x x (256) + 4x h (512)
FN = 512                          # FC token chunk per matmul

f32 = mybir.dt.float32
f32r = mybir.dt.float32r
bf16 = mybir.dt.bfloat16
i32 = mybir.dt.int32

_CACHE = {}


def _build(t_steps=None, rec_repeats=1, fc_repeats=1, do_fc=True,
           debug_dump=False):
    if t_steps is None:
        t_steps = T
    nc = bacc.Bacc("TRN2", target_bir_lowering=False, debug=False,
                   num_devices=N_CORES)

    n_tok_tiles = NTOK // 128
    idx_d = nc.dram_tensor("idx", [n_tok_tiles, 128], i32, kind="ExternalInput").ap()
    emb_d = nc.dram_tensor("emb", [V, E], f32, kind="ExternalInput").ap()
    wcat_d = nc.dram_tensor("wcat", [2, KR, 128, G4], bf16, kind="ExternalInput").ap()
    fcw_d = nc.dram_tensor("fcw", [8, 128, VPAD], bf16, kind="ExternalInput").ap()
    ident_d = nc.dram_tensor("ident", [128, 128], f32, kind="ExternalInput").ap()
    out_d = nc.dram_tensor("logitsT", [VPAD, NTOK], f32, kind="ExternalOutput").ap()
    if debug_dump:
        dbg_xT = nc.dram_tensor("dbg_xT", [2, 128, NTOK], f32,
                                kind="ExternalOutput").ap()
        dbg_g = nc.dram_tensor("dbg_g", [t_steps, 128, 1024], f32,
                               kind="ExternalOutput").ap()
        dbg_h = nc.dram_tensor("dbg_h", [t_steps, 128, 256], f32,
                               kind="ExternalOutput").ap()
        dbg_hT = nc.dram_tensor("dbg_hT", [128, 8 * NTOK], f32,
                                kind="ExternalOutput").ap()

    with tile.TileContext(nc) as tc, ExitStack() as top:
        const_pool = top.enter_context(tc.tile_pool(name="const", bufs=1))
        ident_sb = const_pool.tile([128, 128], f32)
        nc.sync.dma_start(ident_sb[:], ident_d[:])
        idx_sb = const_pool.tile([128, n_tok_tiles], i32)
        nc.sync.dma_start(idx_sb[:], idx_d.rearrange("a b -> b a"))

        # h^T resident store: chunk k = 4*dir + unit_group, at col k*NTOK
        hT_pool = top.enter_context(tc.tile_pool(name="hTp", bufs=1))
        hT_all = hT_pool.tile([128, 8 * NTOK], bf16)

        # fc_w chunks 0-3 prefetched during recurrence, 4-7 at FC start
        fcwA = top.enter_context(tc.tile_pool(name="fcwA", bufs=1))
        fcw_sb = [None] * 8
        for k in range(4):
            fcw_sb[k] = fcwA.tile([128, VPAD], bf16, name=f"fcw{k}")
            nc.sync.dma_start(fcw_sb[k][:], fcw_d[k])

        with ExitStack() as rec_ctx:
            wp = rec_ctx.enter_context(tc.tile_pool(name="wp", bufs=1))
            w_sb = [[None] * KR for _ in range(2)]
            for d in range(2):
                for r in range(KR):
                    w_sb[d][r] = wp.tile([128, G4], bf16, name=f"w{d}_{r}")
                    nc.sync.dma_start(w_sb[d][r][:], wcat_d[d, r])

            xt_pool = rec_ctx.enter_context(tc.tile_pool(name="xt", bufs=1))
            xT = [xt_pool.tile([128, NTOK], bf16, name=f"xT{hf}") for hf in range(2)]

            # ---- gather + transpose x^T (pools closed before recurrence) ----
            with ExitStack() as gctx:
                gat_pool = gctx.enter_context(tc.tile_pool(name="gat", bufs=4))
                gps_pool = gctx.enter_context(
                    tc.tile_pool(name="gps", bufs=4, space="PSUM"))
                for i in range(n_tok_tiles):
                    x_nat = gat_pool.tile([128, E], f32, tag="xnat")
                    nc.gpsimd.indirect_dma_start(
                        out=x_nat[:], out_offset=None, in_=emb_d[:],
                        in_offset=bass.IndirectOffsetOnAxis(
                            ap=idx_sb[:, i:i + 1], axis=0))
                    for hf in range(2):
                        xp = gps_pool.tile([128, 128], f32, tag="xp")
                        nc.tensor.transpose(
                            xp[:], x_nat[:, 128 * hf:128 * (hf + 1)], ident_sb[:])
                        nc.vector.tensor_copy(
                            xT[hf][:, 128 * i:128 * (i + 1)], xp[:])

            # ---- recurrence state ----
            st_pool = rec_ctx.enter_context(tc.tile_pool(name="st", bufs=1))
            hT_zero = st_pool.tile([128, 64], bf16)
            nc.vector.memset(hT_zero[:], 0.0)
            c_sb = st_pool.tile([128, 256], f32)       # [*, d*128 + u]
            nc.vector.memset(c_sb[:], 0.0)

            ps_pool = rec_ctx.enter_context(
                tc.tile_pool(name="rps", bufs=2, space="PSUM"))
            ew_pool = rec_ctx.enter_context(tc.tile_pool(name="ew", bufs=2))

            for rep in range(rec_repeats):
                for s in range(t_steps):
                    g_ps = ps_pool.tile([128, 1024], f32, tag="g")
                    for d in range(2):
                        t = s if d == 0 else T - 1 - s
                        tp_prev = s - 1 if d == 0 else T - s
                        for c4 in range(4):
                            for r in range(KR):
                                if r >= 2:                  # h chunk q = r-2
                                    if s == 0:
                                        lhsT = hT_zero[:, (r - 2) * 16:(r - 1) * 16]
                                    else:
                                        o = (4 * d + r - 2) * NTOK + tp_prev * 16
                                        lhsT = hT_all[:, o:o + 16]
                                else:                       # x chunk
                                    o = t * 16
                                    lhsT = xT[r][:, o:o + 16]
                                nc.tensor.matmul(
                                    g_ps[32 * c4:32 * c4 + 16,
                                         512 * d:512 * (d + 1)],
                                    lhsT,
                                    w_sb[d][r][:, 512 * c4:512 * (c4 + 1)],
                                    start=(r == 0), stop=(r == KR - 1),
                                    tile_position=(0, 32 * c4))

                    if debug_dump and rep == 0:
                        dg = ew_pool.tile([128, 1024], f32, tag="dbgg")
                        nc.scalar.copy(dg[:], g_ps[:])
                        nc.sync.dma_start(dbg_g[s], dg[:])

                    g3 = g_ps[:].rearrange("p (d c) -> p d c", d=2)
                    sig_t = ew_pool.tile([128, 768], f32, tag="sig")
                    nc.scalar.activation(
                        sig_t[:].rearrange("p (d c) -> p d c", d=2),
                        g3[:, :, 0:384], mybir.ActivationFunctionType.Sigmoid)
                    cht_t = ew_pool.tile([128, 256], f32, tag="cht")
                    nc.scalar.activation(
                        cht_t[:].rearrange("p (d c) -> p d c", d=2),
                        g3[:, :, 384:512], mybir.ActivationFunctionType.Tanh)

                    sg3 = sig_t[:].rearrange("p (d c) -> p d c", d=2)
                    tmp1 = ew_pool.tile([128, 256], f32, tag="tmp1")
                    nc.vector.tensor_tensor(tmp1[:], sg3[:, :, 128:256], c_sb[:],
                                            op=mybir.AluOpType.mult)
                    tmp2 = ew_pool.tile([128, 256], f32, tag="tmp2")
                    nc.vector.tensor_tensor(tmp2[:], sg3[:, :, 0:128], cht_t[:],
                                            op=mybir.AluOpType.mult)
                    nc.vector.tensor_tensor(c_sb[:], tmp1[:], tmp2[:],
                                            op=mybir.AluOpType.add)
                    tanc = ew_pool.tile([128, 256], f32, tag="tanc")
                    nc.scalar.activation(tanc[:], c_sb[:],
                                         mybir.ActivationFunctionType.Tanh)
                    h_t = ew_pool.tile([128, 256], f32, tag="ht")
                    nc.vector.tensor_tensor(h_t[:], sg3[:, :, 256:384], tanc[:],
                                            op=mybir.AluOpType.mult)

                    for d in range(2):
                        t = s if d == 0 else T - 1 - s
                        tp = ps_pool.tile([128, 128], f32, tag=f"tp{d}")
                        nc.tensor.transpose(
                            tp[:], h_t[:, 128 * d:128 * (d + 1)], ident_sb[:])
                        # tp cols 32q..32q+16 hold h^T of unit-group q
                        dst = hT_all[:].rearrange("p (k n) -> p k n", k=8)[
                            :, 4 * d:4 * d + 4, t * 16:t * 16 + 16]
                        nc.vector.tensor_copy(
                            dst,
                            tp[:].rearrange("p (q c) -> p q c", q=4)[:, :, 0:16])
                    if debug_dump and rep == 0:
                        nc.sync.dma_start(dbg_h[s], h_t[:])
            if debug_dump:
                for hf in range(2):
                    dx = ew_pool.tile([128, NTOK], f32, tag="dbgx")
                    nc.vector.tensor_copy(dx[:], xT[hf][:])
                    nc.sync.dma_start(dbg_xT[hf], dx[:])
                for k in range(8):
                    dh = ew_pool.tile([128, NTOK], f32, tag="dbgx")
                    nc.vector.tensor_copy(
                        dh[:], hT_all[:, k * NTOK:(k + 1) * NTOK])
                    nc.sync.dma_start(dbg_hT[:, k * NTOK:(k + 1) * NTOK], dh[:])
            # end recurrence

        if do_fc:
            with ExitStack() as fc_ctx:
                fcwB = fc_ctx.enter_context(tc.tile_pool(name="fcwB", bufs=1))
                for k in range(4, 8):
                    fcw_sb[k] = fcwB.tile([128, VPAD], bf16, name=f"fcw{k}")
                    nc.sync.dma_start(fcw_sb[k][:], fcw_d[k])
                fps_pool = fc_ctx.enter_context(
                    tc.tile_pool(name="fps", bufs=2, space="PSUM"))
                ev_pool = fc_ctx.enter_context(tc.tile_pool(name="ev", bufs=3))

                fn = min(FN, NTOK)
                for rep in range(fc_repeats):
                    for v in range(VPAD // 128):
                        for n in range(NTOK // fn):
                            pf = fps_pool.tile([128, fn], f32, tag=f"pf{n % 4}")
                            for k in range(8):
                                nc.tensor.matmul(
                                    pf[:], fcw_sb[k][:, 128 * v:128 * (v + 1)],
                                    hT_all[:, k * NTOK + fn * n:
                                           k * NTOK + fn * (n + 1)],
                                    start=(k == 0), stop=(k == 7))
                            ev = ev_pool.tile([128, fn], f32, tag=f"ev{n % 2}")
                            if n % 2 == 0:
                                nc.vector.tensor_copy(ev[:], pf[:])
                            else:
                                nc.scalar.copy(ev[:], pf[:])
                            nc.sync.dma_start(
                                out_d[128 * v:128 * (v + 1),
                                      fn * n:fn * (n + 1)], ev[:])

    nc.compile()
    return nc


def _host_prep(inputs, emb, Wh_fwd, Wx_fwd, b_fwd, Wh_bwd, Wx_bwd, b_bwd,
               fc_w, fc_b):
    idx = np.ascontiguousarray(
        np.asarray(inputs).astype(np.int32).T.reshape(NTOK // 128, 128))
    emb = np.ascontiguousarray(np.asarray(emb, dtype=np.float32))

    wcat = np.zeros((2, KR, 128, G4), dtype=np.float32)
    cols = (np.arange(H) // 128) * 512 + (np.arange(H) % 128)
    for d, (Wh, Wx) in enumerate(((Wh_fwd, Wx_fwd), (Wh_bwd, Wx_bwd))):
        Wh = np.asarray(Wh, dtype=np.float32)   # [4, H, H]
        Wx = np.asarray(Wx, dtype=np.float32)   # [4, E, H]
        Wfull = np.zeros((E + H, G4), dtype=np.float32)   # rows: x then h
        for g in range(4):
            Wfull[:E, cols + g * 128] = Wx[g]
            Wfull[E:, cols + g * 128] = Wh[g]
        wcat[d] = Wfull.reshape(KR, 128, G4)
    wcat = wcat.astype(ml_dtypes.bfloat16)

    fc_w = np.asarray(fc_w, dtype=np.float32)
    fcw_shards = []
    for c in range(N_CORES):
        sh = np.zeros((2 * H, VPAD), dtype=np.float32)
        sh[:, :VSH] = fc_w[:, c * VSH:(c + 1) * VSH]
        fcw_shards.append(np.ascontiguousarray(
            sh.reshape(8, 128, VPAD).astype(ml_dtypes.bfloat16)))

    ident = np.eye(128, dtype=np.float32)
    base = {"idx": idx, "emb": emb, "wcat": wcat, "ident": ident}
    in_maps = [dict(base, fcw=fcw_shards[c]) for c in range(N_CORES)]
    lstm_bias_zero = (not np.any(np.asarray(b_fwd))) and \
        (not np.any(np.asarray(b_bwd)))
    return in_maps, lstm_bias_zero


def run(in_maps, nc=None, **build_kw):
    if nc is None:
        key = tuple(sorted(build_kw.items()))
        if key not in _CACHE:
            _CACHE[key] = _build(**build_kw)
        nc = _CACHE[key]
    res = run_bass_kernel_spmd(nc, in_maps, core_ids=list(range(N_CORES)))
    return res


def kernel(**inputs):
    in_maps, lstm_bias_zero = _host_prep(**inputs)
    assert lstm_bias_zero, "nonzero LSTM biases not supported by this build"
    res = run(in_maps)
    parts = [res.results[c]["logitsT"][:VSH] for c in range(N_CORES)]
    logitsT = np.concatenate(parts, axis=0)          # [V, NTOK]
    out = logitsT.T.reshape(T, B, V).transpose(1, 0, 2)
    out = np.ascontiguousarray(out, dtype=np.float32)
    fc_b = np.asarray(inputs["fc_b"], dtype=np.float32)
    if np.any(fc_b):
        out += fc_b
    return out
